# revision 15
# baseline (speedup 1.0000x reference)
"""GCN encoder (2-layer) + inner-product decoder on 8 Trainium2 NeuronCores.

Computation (reference):
    support1 = x @ W1                  [N, 256]
    h1  = relu(A @ support1 + b1)      A[dst,src] = sum of edge weights
    support2 = h1 @ W2                 [N, 64]
    z   = relu(A @ support2 + b2)
    recon = sigmoid(z @ z.T)           [N, N]

Strategy: dst-node row-sharding across 8 cores. The sparse aggregation is
done as a dense fp16 matmul against the (transposed) adjacency matrix,
streamed from HBM. Layouts are chosen so every matmul's stationary operand
(lhsT) is produced directly by the previous stage — no on-device transposes:

    stage A (all cores, replicated): support1 = x @ W1 (from host-transposed xT)
    stage B: h1T[f1, dst_shard] = relu(support1.T-contract vs AT + b1)
    stage C: support2_shard = h1 @ W2 -> AllGather -> support2 full
    stage D: zT[f2, dst_shard] = relu(... vs AT + b2) -> AllGather zT
    stage E: recon_shard = sigmoid(z_shard @ z_full.T) -> HBM

All matmul inputs fp16 (fp32 PSUM accumulation); outputs fp32.
"""

import numpy as np

N = 10000
NPAD = 10240
NCORES = 8
SHARD = NPAD // NCORES  # 1280
P = 128
F0, F1, F2 = 512, 256, 64
KC = NPAD // P          # 80 src chunks
KSLAB = 20              # src chunks per streamed AT slab
NSLABS = KC // KSLAB    # 4
NT = [(0, 512), (512, 512), (1024, 256)]  # dst tiles within a shard
AT_FREE = sum(KSLAB * nt * NSLABS for (_, nt) in NT)  # 102400 per partition
DBLK = 41               # decoder col blocks per row block (circulant triangle)
WIN = DBLK * P          # 5248 decoder cols per row tile

_compiled_nc = None


def _build_program():
    import concourse.bass as bass
    import concourse.bacc as bacc
    import concourse.mybir as mybir
    import concourse.tile as tile

    f16 = mybir.dt.float16
    f32 = mybir.dt.float32
    AF = mybir.ActivationFunctionType

    nc = bacc.Bacc("TRN2", target_bir_lowering=False, debug=False,
                   num_devices=NCORES)

    # ---- I/O ----
    xT_h = nc.dram_tensor("xT", [F0, NPAD], f16, kind="ExternalInput")
    atb_h = nc.dram_tensor("ATB", [P, AT_FREE], f16, kind="ExternalInput")
    atd_h = nc.dram_tensor("ATD", [P, AT_FREE], f16, kind="ExternalInput")
    w1_h = nc.dram_tensor("W1", [F0, F1], f16, kind="ExternalInput")
    w2_h = nc.dram_tensor("W2", [F1, F2], f16, kind="ExternalInput")
    b1_h = nc.dram_tensor("b1", [F1, 1], f32, kind="ExternalInput")
    b2_h = nc.dram_tensor("b2", [F2, 1], f32, kind="ExternalInput")
    zsel_h = nc.dram_tensor("zsel", [F2, NCORES], mybir.dt.int32,
                            kind="ExternalInput")
    zT_out = nc.dram_tensor("zT_out", [F2, SHARD], f32, kind="ExternalOutput")
    recon_out = nc.dram_tensor("recon_out", [SHARD, WIN], f16,
                               kind="ExternalOutput")

    groups = [list(range(NCORES))]

    with tile.TileContext(nc) as tc:
        with (
            tc.tile_pool(name="persist", bufs=1) as pp,
            tc.tile_pool(name="dram", bufs=1, space="DRAM") as dram,
        ):
            w1_sb = pp.tile([P, 4, F1], f16)
            nc.sync.dma_start(w1_sb[:], w1_h[:].rearrange("(c p) f -> p c f", p=P))
            w2_sb = pp.tile([P, 2, F2], f16)
            nc.sync.dma_start(w2_sb[:], w2_h[:].rearrange("(c p) f -> p c f", p=P))
            b1_sb = pp.tile([P, 2, 1], f32)
            nc.sync.dma_start(b1_sb[:], b1_h[:].rearrange("(c p) f -> p c f", p=P))
            b2_sb = pp.tile([F2, 1], f32)
            nc.sync.dma_start(b2_sb[:], b2_h[:])

            # tiny early collective: pays the CC-stream init cost during
            # stage A so the real collectives start with ~1us trigger delay
            wtile = pp.tile([16, 16], f16)
            nc.gpsimd.memset(wtile[:], 0.0)
            warm_in = dram.tile([16, 16], f16)
            warm_out = dram.tile([NCORES, 16, 16], f16)
            nc.gpsimd.dma_start(warm_in[:], wtile[:])
            nc.gpsimd.collective_compute(
                "AllGather", mybir.AluOpType.bypass, replica_groups=groups,
                ins=[warm_in.opt()], outs=[warm_out.opt()])

            s1_sb = pp.tile([P, KC * F1], f16)        # support1 full, 40KB/part
            h1T_sb = pp.tile([P, 2 * SHARD], f16)     # h1 transposed (f1 on parts)
            s2full_sb = pp.tile([P, KC, F2], f16)     # support2 full (perm M2)
            zT_sb = pp.tile([F2, SHARD], f32)
            zT16_sb = pp.tile([F2, SHARD], f16)
            zTfull_sb = pp.tile([F2, NCORES, SHARD], f16)

            # ---- stage A: support1 = x @ W1 (replicated, all NPAD rows) ----
            with (
                tc.tile_pool(name="xT", bufs=1) as xp,
                tc.tile_pool(name="psA", bufs=4, space="PSUM") as psA,
            ):
                xT_sb = xp.tile([P, 4, NPAD], f16)   # 80KB/part
                xT_view = xT_h[:].rearrange("(c p) n -> p c n", p=P)
                for nb in range(8):
                    nc.sync.dma_start(
                        xT_sb[:, :, nb * 1280:(nb + 1) * 1280],
                        xT_view[:, :, nb * 1280:(nb + 1) * 1280])
                for t in range(KC):
                    ps = psA.tile([P, F1], f32, tag="psA")
                    for c in range(4):
                        nc.tensor.matmul(
                            ps[:],
                            xT_sb[:, c, t * P:(t + 1) * P],
                            w1_sb[:, c, :],
                            start=(c == 0), stop=(c == 3))
                    nc.vector.tensor_copy(s1_sb[:, t * F1:(t + 1) * F1], ps[:])

            # ---- stage B: h1T = relu(contract(support1, AT) + b1) ----
            atoff = 0
            with (
                tc.tile_pool(name="atB", bufs=3) as ab,
                tc.tile_pool(name="psB", bufs=2, space="PSUM") as psB,
            ):
                for (n_off, nt) in NT:
                    psb = [psB.tile([P, nt], f32, tag=f"psB{m}", name=f"psB{m}")
                           for m in range(2)]
                    for kb in range(NSLABS):
                        at = ab.tile([P, KSLAB * 512], f16, tag="atB")
                        nc.sync.dma_start(
                            at[:, :KSLAB * nt],
                            atb_h[:, atoff: atoff + KSLAB * nt])
                        atoff += KSLAB * nt
                        for kk in range(KSLAB):
                            k = kb * KSLAB + kk
                            for m in range(2):
                                nc.tensor.matmul(
                                    psb[m][:],
                                    s1_sb[:, k * F1 + m * P: k * F1 + (m + 1) * P],
                                    at[:, kk * nt:(kk + 1) * nt],
                                    start=(k == 0), stop=(k == KC - 1))
                    for m in range(2):
                        nc.scalar.activation(
                            h1T_sb[:, m * SHARD + n_off: m * SHARD + n_off + nt],
                            psb[m][:], AF.Relu, bias=b1_sb[:, m, :])

            # ---- stage C: support2 = h1 @ W2 ; AllGather ----
            s2_sb = pp.tile([P, SHARD // P, F2], f16)
            with tc.tile_pool(name="psC", bufs=4, space="PSUM") as psC:
                for t in range(SHARD // P):
                    ps = psC.tile([P, F2], f32, tag="psC")
                    for m in range(2):
                        nc.tensor.matmul(
                            ps[:],
                            h1T_sb[:, m * SHARD + t * P: m * SHARD + (t + 1) * P],
                            w2_sb[:, m, :],
                            start=(m == 0), stop=(m == 1))
                    nc.vector.tensor_copy(s2_sb[:, t, :], ps[:])
            # two AllGathers over k-ranges so stage D can contract the first
            # src half while the second half is still gathering.
            # M2 perm: partition p = 16r + a holds node 1280r + 640j + 40a + kk
            # for k = 40j + kk  ->  k-chunk j is a contiguous 640-row range of
            # every core's shard, and each lhsT k-slice spans all partitions.
            for j in range(2):
                s2b_in = dram.tile([SHARD // 2, F2], f16, name=f"s2bi{j}")
                s2b_out = dram.tile([NCORES, SHARD // 2, F2], f16,
                                    name=f"s2bo{j}")
                nc.gpsimd.dma_start(
                    s2b_in[:].rearrange("(t p) f -> p t f", p=P),
                    s2_sb[:, 5 * j:5 * (j + 1), :])
                nc.gpsimd.collective_compute(
                    "AllGather", mybir.AluOpType.bypass, replica_groups=groups,
                    ins=[s2b_in.opt()], outs=[s2b_out.opt()])
                nc.scalar.dma_start(
                    s2full_sb[:, 40 * j:40 * (j + 1), :],
                    s2b_out[:].rearrange("r (a kk) f -> (r a) kk f", a=16))

            # ---- stage D: zT = relu(contract(support2, AT) + b2); AllGather ----
            atoff = 0
            with (
                tc.tile_pool(name="atD", bufs=3) as ad,
                tc.tile_pool(name="psD", bufs=2, space="PSUM") as psD,
            ):
                for (n_off, nt) in NT:
                    psd = psD.tile([F2, nt], f32, tag="psD")
                    for kb in range(NSLABS):
                        at = ad.tile([P, KSLAB * 512], f16, tag="atD")
                        nc.sync.dma_start(
                            at[:, :KSLAB * nt],
                            atd_h[:, atoff: atoff + KSLAB * nt])
                        atoff += KSLAB * nt
                        for kk in range(KSLAB):
                            k = kb * KSLAB + kk
                            nc.tensor.matmul(
                                psd[:],
                                s2full_sb[:, k, :],
                                at[:, kk * nt:(kk + 1) * nt],
                                start=(k == 0), stop=(k == KC - 1))
                    nc.scalar.activation(
                        zT_sb[:, n_off:n_off + nt], psd[:], AF.Relu,
                        bias=b2_sb[:, 0:1])
                    nc.vector.tensor_copy(
                        zT16_sb[:, n_off:n_off + nt], zT_sb[:, n_off:n_off + nt])
            nc.scalar.dma_start(zT_out[:], zT_sb[:])
            zsel_sb = pp.tile([F2, NCORES], mybir.dt.int32)
            nc.scalar.dma_start(zsel_sb[:], zsel_h[:])
            # per-n-tile chunked AllGather of zT16, then rotated gather:
            # zTfull chunk p holds rank (pid + p) % 8 (host-built zsel).
            for (n_off, nt) in NT:
                zb_in = dram.tile([F2, nt], f16, name=f"zb_in{n_off}")
                zb_out = dram.tile([NCORES, F2, nt], f16,
                                   name=f"zb_out{n_off}")
                nc.gpsimd.dma_start(zb_in[:], zT16_sb[:, n_off:n_off + nt])
                nc.gpsimd.collective_compute(
                    "AllGather", mybir.AluOpType.bypass, replica_groups=groups,
                    ins=[zb_in.opt()], outs=[zb_out.opt()])
                zb_rows = zb_out[:].rearrange("r f n -> (r f) n")
                for p in range(NCORES):
                    nc.gpsimd.indirect_dma_start(
                        out=zTfull_sb[:, p, n_off:n_off + nt],
                        out_offset=None,
                        in_=zb_rows,
                        in_offset=bass.IndirectOffsetOnAxis(
                            ap=zsel_sb[:, p:p + 1], axis=0),
                    )

            # ---- stage E: recon upper-circulant = sigmoid(z @ z_rot.T) ----
            zTflat = zTfull_sb[:].rearrange("f r n -> f (r n)")
            with (
                tc.tile_pool(name="sig", bufs=2) as sp,
                tc.tile_pool(name="psE", bufs=4, space="PSUM") as psE,
            ):
                for mt in range(SHARD // P):
                    logit = sp.tile([P, WIN], f16, tag="logit")
                    sig = sp.tile([P, WIN], f16, tag="sig")
                    base = mt * P
                    done = 0
                    while done < WIN:
                        seg = min(512, WIN - done)
                        ps = psE.tile([P, 512], f32, tag="psE")
                        nc.tensor.matmul(
                            ps[:, :seg],
                            zT16_sb[:, mt * P:(mt + 1) * P],
                            zTflat[:, base + done: base + done + seg],
                            start=True, stop=True)
                        nc.vector.tensor_copy(
                            logit[:, done:done + seg], ps[:, :seg])
                        done += seg
                    nc.scalar.activation(sig[:], logit[:], AF.Sigmoid)
                    nc.sync.dma_start(
                        recon_out[mt * P:(mt + 1) * P, :], sig[:])

    nc.compile()
    return nc


def _get_program():
    global _compiled_nc
    if _compiled_nc is None:
        _compiled_nc = _build_program()
    return _compiled_nc


def _host_prep(x, W1, b1, W2, b2, edge_weight, edge_src, edge_dst):
    x = np.asarray(x, dtype=np.float32)
    W1 = np.asarray(W1, dtype=np.float32)
    W2 = np.asarray(W2, dtype=np.float32)
    b1 = np.asarray(b1, dtype=np.float32)
    b2 = np.asarray(b2, dtype=np.float32)
    edge_weight = np.asarray(edge_weight, dtype=np.float32)
    edge_src = np.asarray(edge_src, dtype=np.int64)
    edge_dst = np.asarray(edge_dst, dtype=np.int64)

    # x^T padded, fp16
    xT = np.zeros((F0, NPAD), dtype=np.float16)
    xT[:, :N] = x.T.astype(np.float16)

    # dense transposed adjacency AT[src, dst] = sum of w over parallel edges
    AT = np.zeros((NPAD, NPAD), dtype=np.float32)
    np.add.at(AT, (edge_src, edge_dst), edge_weight)
    AT16 = AT.astype(np.float16)
    del AT

    in_maps = []
    common = {
        "xT": xT,
        "W1": W1.astype(np.float16),
        "W2": W2.astype(np.float16),
        "b1": b1.reshape(F1, 1).astype(np.float32),
        "b2": b2.reshape(F2, 1).astype(np.float32),
    }
    for c in range(NCORES):
        zsel = np.zeros((F2, NCORES), np.int32)
        for f in range(F2):
            for p in range(NCORES):
                zsel[f, p] = ((c + p) % NCORES) * F2 + f
        atc = AT16[:, c * SHARD:(c + 1) * SHARD]  # [NPAD, SHARD]
        # ATB: src perm M1 (node = k*128 + p), slab-major layout
        slabs_b = []
        slabs_d = []
        # M2 perm: node(p, k) = 1280*(p//16) + 640*(k//40) + 40*(p%16) + k%40
        pg = np.arange(P)[:, None]
        kg = np.arange(KC)[None, :]
        m2_rows = 1280 * (pg // 16) + 640 * (kg // 40) + 40 * (pg % 16) + kg % 40
        atc_m2 = atc[m2_rows]  # [p, k, dst]
        for (n_off, nt) in NT:
            blkn = atc[:, n_off:n_off + nt]
            for kb in range(NSLABS):
                blk = blkn[kb * KSLAB * P:(kb + 1) * KSLAB * P, :]
                slabs_b.append(
                    blk.reshape(KSLAB, P, nt).transpose(1, 0, 2).reshape(P, -1))
                slabs_d.append(
                    atc_m2[:, kb * KSLAB:(kb + 1) * KSLAB, n_off:n_off + nt]
                    .reshape(P, -1))
        in_map = dict(common)
        in_map["zsel"] = zsel
        in_map["ATB"] = np.ascontiguousarray(np.concatenate(slabs_b, axis=1))
        in_map["ATD"] = np.ascontiguousarray(np.concatenate(slabs_d, axis=1))
        in_maps.append(in_map)
    return in_maps


def kernel(x, W1, b1, W2, b2, edge_weight, edge_src, edge_dst):
    from concourse import bass_utils

    nc = _get_program()
    in_maps = _host_prep(x, W1, b1, W2, b2, edge_weight, edge_src, edge_dst)
    res = bass_utils.run_bass_kernel_spmd(
        nc, in_maps, core_ids=list(range(NCORES)))
    z = np.concatenate(
        [res.results[c]["zT_out"].T for c in range(NCORES)], axis=0)[:N]

    M = np.zeros((NPAD, NPAD), dtype=np.float32)
    NBLK = NPAD // P  # 80
    for c in range(NCORES):
        ro = res.results[c]["recon_out"].astype(np.float32)  # [SHARD, WIN]
        for mt in range(SHARD // P):
            gI = c * (SHARD // P) + mt
            rows = slice(gI * P, (gI + 1) * P)
            strip = ro[mt * P:(mt + 1) * P, :]
            g0 = (gI * P) % NPAD
            if g0 + WIN <= NPAD:
                M[rows, g0:g0 + WIN] = strip
            else:
                k = NPAD - g0
                M[rows, g0:] = strip[:, :k]
                M[rows, :WIN - k] = strip[:, k:]
    # mirror the circulant upper coverage onto the lower blocks
    Mb = M.reshape(NBLK, P, NBLK, P)
    Is, Js = [], []
    for I in range(NBLK):
        for dd in range(1, DBLK):
            Is.append(I)
            Js.append((I + dd) % NBLK)
    Isa = np.asarray(Is)
    Jsa = np.asarray(Js)
    Mb[Jsa, :, Isa, :] = Mb[Isa, :, Jsa, :].transpose(0, 2, 1)
    recon = M[:N, :N]
    return z.astype(np.float32), np.ascontiguousarray(recon)


# revision 17
# speedup vs baseline: 1.1305x; 1.1305x over previous
"""GCN encoder (2-layer) + inner-product decoder on 8 Trainium2 NeuronCores.

Computation (reference):
    support1 = x @ W1                  [N, 256]
    h1  = relu(A @ support1 + b1)      A[dst,src] = sum of edge weights
    support2 = h1 @ W2                 [N, 64]
    z   = relu(A @ support2 + b2)
    recon = sigmoid(z @ z.T)           [N, N]

Strategy: dst-node row-sharding across 8 cores. The sparse aggregation is
done as a dense fp16 matmul against the (transposed) adjacency matrix,
streamed from HBM. Layouts are chosen so every matmul's stationary operand
(lhsT) is produced directly by the previous stage — no on-device transposes:

    stage A (all cores, replicated): support1 = x @ W1 (from host-transposed xT)
    stage B: h1T[f1, dst_shard] = relu(support1.T-contract vs AT + b1)
    stage C: support2_shard = h1 @ W2 -> AllGather -> support2 full
    stage D: zT[f2, dst_shard] = relu(... vs AT + b2) -> AllGather zT
    stage E: recon_shard = sigmoid(z_shard @ z_full.T) -> HBM

All matmul inputs fp16 (fp32 PSUM accumulation); outputs fp32.
"""

import numpy as np

N = 10000
NPAD = 10240
NCORES = 8
SHARD = NPAD // NCORES  # 1280
P = 128
F0, F1, F2 = 512, 256, 64
KC = NPAD // P          # 80 src chunks
KSLAB = 20              # src chunks per streamed AT slab
NSLABS = KC // KSLAB    # 4
NT = [(0, 512), (512, 512), (1024, 256)]  # dst tiles within a shard
AT_FREE = sum(KSLAB * nt * NSLABS for (_, nt) in NT)  # 102400 per partition
DBLK = 41               # decoder col blocks per row block (circulant triangle)
WIN = DBLK * P          # 5248 decoder cols per row tile

_compiled_nc = None


def _build_program():
    import concourse.bass as bass
    import concourse.bacc as bacc
    import concourse.mybir as mybir
    import concourse.tile as tile

    f16 = mybir.dt.float16
    f32 = mybir.dt.float32
    AF = mybir.ActivationFunctionType

    nc = bacc.Bacc("TRN2", target_bir_lowering=False, debug=False,
                   num_devices=NCORES)

    # ---- I/O ----
    xT_h = nc.dram_tensor("xT", [F0, NPAD], f16, kind="ExternalInput")
    atb_h = nc.dram_tensor("ATB", [P, AT_FREE], f16, kind="ExternalInput")
    atd_h = nc.dram_tensor("ATD", [P, AT_FREE], f16, kind="ExternalInput")
    w1_h = nc.dram_tensor("W1", [F0, F1], f16, kind="ExternalInput")
    w2_h = nc.dram_tensor("W2", [F1, F2], f16, kind="ExternalInput")
    b1_h = nc.dram_tensor("b1", [F1, 1], f32, kind="ExternalInput")
    b2_h = nc.dram_tensor("b2", [F2, 1], f32, kind="ExternalInput")
    zsel_h = nc.dram_tensor("zsel", [F2, NCORES], mybir.dt.int32,
                            kind="ExternalInput")
    zT_out = nc.dram_tensor("zT_out", [F2, SHARD], f32, kind="ExternalOutput")
    recon_out = nc.dram_tensor("recon_out", [SHARD, WIN], f16,
                               kind="ExternalOutput")

    groups = [list(range(NCORES))]

    with tile.TileContext(nc) as tc:
        with (
            tc.tile_pool(name="persist", bufs=1) as pp,
            tc.tile_pool(name="dram", bufs=1, space="DRAM") as dram,
        ):
            w1_sb = pp.tile([P, 4, F1], f16)
            nc.sync.dma_start(w1_sb[:], w1_h[:].rearrange("(c p) f -> p c f", p=P))
            w2_sb = pp.tile([P, 2, F2], f16)
            nc.sync.dma_start(w2_sb[:], w2_h[:].rearrange("(c p) f -> p c f", p=P))
            b1_sb = pp.tile([P, 2, 1], f32)
            nc.sync.dma_start(b1_sb[:], b1_h[:].rearrange("(c p) f -> p c f", p=P))
            b2_sb = pp.tile([F2, 1], f32)
            nc.sync.dma_start(b2_sb[:], b2_h[:])

            # tiny early collective: pays the CC-stream init cost during
            # stage A so the real collectives start with ~1us trigger delay
            wtile = pp.tile([16, 16], f16)
            nc.gpsimd.memset(wtile[:], 0.0)
            warm_in = dram.tile([16, 16], f16)
            warm_out = dram.tile([NCORES, 16, 16], f16)
            nc.gpsimd.dma_start(warm_in[:], wtile[:])
            nc.gpsimd.collective_compute(
                "AllGather", mybir.AluOpType.bypass, replica_groups=groups,
                ins=[warm_in.opt()], outs=[warm_out.opt()])

            s1_sb = pp.tile([P, KC * F1], f16)        # support1 full, 40KB/part
            h1T_sb = pp.tile([P, 2 * SHARD], f16)     # h1 transposed (f1 on parts)
            s2full_sb = pp.tile([P, KC, F2], f16)     # support2 full (perm M2)
            zT_sb = pp.tile([F2, SHARD], f32)
            zT16_sb = pp.tile([F2, SHARD], f16)
            zTfull_sb = pp.tile([F2, NCORES, SHARD], f16)

            # ---- stage A: support1 = x @ W1 (replicated, all NPAD rows) ----
            with (
                tc.tile_pool(name="xT", bufs=1) as xp,
                tc.tile_pool(name="psA", bufs=4, space="PSUM") as psA,
            ):
                xT_sb = xp.tile([P, 4, NPAD], f16)   # 80KB/part
                xT_view = xT_h[:].rearrange("(c p) n -> p c n", p=P)
                for nb in range(8):
                    nc.sync.dma_start(
                        xT_sb[:, :, nb * 1280:(nb + 1) * 1280],
                        xT_view[:, :, nb * 1280:(nb + 1) * 1280])
                for t in range(KC):
                    ps = psA.tile([P, F1], f32, tag="psA")
                    for c in range(4):
                        nc.tensor.matmul(
                            ps[:],
                            xT_sb[:, c, t * P:(t + 1) * P],
                            w1_sb[:, c, :],
                            start=(c == 0), stop=(c == 3))
                    nc.vector.tensor_copy(s1_sb[:, t * F1:(t + 1) * F1], ps[:])

            # ---- stage B: h1T = relu(contract(support1, AT) + b1) ----
            atoff = 0
            with (
                tc.tile_pool(name="atB", bufs=3) as ab,
                tc.tile_pool(name="psB", bufs=2, space="PSUM") as psB,
            ):
                for (n_off, nt) in NT:
                    psb = [psB.tile([P, nt], f32, tag=f"psB{m}", name=f"psB{m}")
                           for m in range(2)]
                    for kb in range(NSLABS):
                        at = ab.tile([P, KSLAB * 512], f16, tag="atB")
                        nc.sync.dma_start(
                            at[:, :KSLAB * nt],
                            atb_h[:, atoff: atoff + KSLAB * nt])
                        atoff += KSLAB * nt
                        for kk in range(KSLAB):
                            k = kb * KSLAB + kk
                            for m in range(2):
                                nc.tensor.matmul(
                                    psb[m][:],
                                    s1_sb[:, k * F1 + m * P: k * F1 + (m + 1) * P],
                                    at[:, kk * nt:(kk + 1) * nt],
                                    start=(k == 0), stop=(k == KC - 1))
                    for m in range(2):
                        nc.scalar.activation(
                            h1T_sb[:, m * SHARD + n_off: m * SHARD + n_off + nt],
                            psb[m][:], AF.Relu, bias=b1_sb[:, m, :])

            # ---- stage C: support2 = h1 @ W2 ; AllGather ----
            s2_sb = pp.tile([P, SHARD // P, F2], f16)
            with tc.tile_pool(name="psC", bufs=4, space="PSUM") as psC:
                for t in range(SHARD // P):
                    ps = psC.tile([P, F2], f32, tag="psC")
                    for m in range(2):
                        nc.tensor.matmul(
                            ps[:],
                            h1T_sb[:, m * SHARD + t * P: m * SHARD + (t + 1) * P],
                            w2_sb[:, m, :],
                            start=(m == 0), stop=(m == 1))
                    nc.vector.tensor_copy(s2_sb[:, t, :], ps[:])
            # single AllGather for support2.
            # M2 perm: partition p = 16r + a holds nodes 1280r + 80a + k
            s2b_in = dram.tile([SHARD, F2], f16)
            s2b_out = dram.tile([NCORES, SHARD, F2], f16)
            nc.gpsimd.dma_start(
                s2b_in[:].rearrange("(t p) f -> p t f", p=P), s2_sb[:])
            nc.gpsimd.collective_compute(
                "AllGather", mybir.AluOpType.bypass, replica_groups=groups,
                ins=[s2b_in.opt()], outs=[s2b_out.opt()])
            nc.scalar.dma_start(
                s2full_sb[:],
                s2b_out[:].rearrange("r (a b) f -> (r a) b f", a=NCORES * 2))

            # ---- stage D: zT = relu(contract(support2, AT) + b2); AllGather ----
            atoff = 0
            with (
                tc.tile_pool(name="atD", bufs=3) as ad,
                tc.tile_pool(name="psD", bufs=2, space="PSUM") as psD,
            ):
                for (n_off, nt) in NT:
                    psd = psD.tile([F2, nt], f32, tag="psD")
                    for kb in range(NSLABS):
                        at = ad.tile([P, KSLAB * 512], f16, tag="atD")
                        nc.sync.dma_start(
                            at[:, :KSLAB * nt],
                            atd_h[:, atoff: atoff + KSLAB * nt])
                        atoff += KSLAB * nt
                        for kk in range(KSLAB):
                            k = kb * KSLAB + kk
                            nc.tensor.matmul(
                                psd[:],
                                s2full_sb[:, k, :],
                                at[:, kk * nt:(kk + 1) * nt],
                                start=(k == 0), stop=(k == KC - 1))
                    nc.scalar.activation(
                        zT_sb[:, n_off:n_off + nt], psd[:], AF.Relu,
                        bias=b2_sb[:, 0:1])
                    nc.vector.tensor_copy(
                        zT16_sb[:, n_off:n_off + nt], zT_sb[:, n_off:n_off + nt])
            nc.scalar.dma_start(zT_out[:], zT_sb[:])
            zsel_sb = pp.tile([F2, NCORES], mybir.dt.int32)
            nc.scalar.dma_start(zsel_sb[:], zsel_h[:])
            # per-n-tile chunked AllGather of zT16, then rotated gather:
            # zTfull chunk p holds rank (pid + p) % 8 (host-built zsel).
            for (n_off, nt) in NT:
                zb_in = dram.tile([F2, nt], f16, name=f"zb_in{n_off}")
                zb_out = dram.tile([NCORES, F2, nt], f16,
                                   name=f"zb_out{n_off}")
                nc.gpsimd.dma_start(zb_in[:], zT16_sb[:, n_off:n_off + nt])
                nc.gpsimd.collective_compute(
                    "AllGather", mybir.AluOpType.bypass, replica_groups=groups,
                    ins=[zb_in.opt()], outs=[zb_out.opt()])
                zb_rows = zb_out[:].rearrange("r f n -> (r f) n")
                for p in range(1, NCORES):
                    nc.gpsimd.indirect_dma_start(
                        out=zTfull_sb[:, p, n_off:n_off + nt],
                        out_offset=None,
                        in_=zb_rows,
                        in_offset=bass.IndirectOffsetOnAxis(
                            ap=zsel_sb[:, p:p + 1], axis=0),
                    )
            # rotated chunk 0 is this core's own zT -> local copy, no
            # collective dependency (lets stage E start on local columns)
            nc.vector.tensor_copy(zTfull_sb[:, 0, :], zT16_sb[:])

            # ---- stage E: recon upper-circulant = sigmoid(z @ z_rot.T) ----
            zTflat = zTfull_sb[:].rearrange("f r n -> f (r n)")
            with (
                tc.tile_pool(name="sig", bufs=2) as sp,
                tc.tile_pool(name="psE", bufs=4, space="PSUM") as psE,
            ):
                for mt in range(SHARD // P):
                    logit = sp.tile([P, WIN], f16, tag="logit")
                    sig = sp.tile([P, WIN], f16, tag="sig")
                    base = mt * P
                    done = 0
                    while done < WIN:
                        seg = min(512, WIN - done)
                        ps = psE.tile([P, 512], f32, tag="psE")
                        nc.tensor.matmul(
                            ps[:, :seg],
                            zT16_sb[:, mt * P:(mt + 1) * P],
                            zTflat[:, base + done: base + done + seg],
                            start=True, stop=True)
                        nc.vector.tensor_copy(
                            logit[:, done:done + seg], ps[:, :seg])
                        done += seg
                    nc.scalar.activation(sig[:], logit[:], AF.Sigmoid)
                    nc.sync.dma_start(
                        recon_out[mt * P:(mt + 1) * P, :], sig[:])

    nc.compile()
    return nc


def _get_program():
    global _compiled_nc
    if _compiled_nc is None:
        _compiled_nc = _build_program()
    return _compiled_nc


def _host_prep(x, W1, b1, W2, b2, edge_weight, edge_src, edge_dst):
    x = np.asarray(x, dtype=np.float32)
    W1 = np.asarray(W1, dtype=np.float32)
    W2 = np.asarray(W2, dtype=np.float32)
    b1 = np.asarray(b1, dtype=np.float32)
    b2 = np.asarray(b2, dtype=np.float32)
    edge_weight = np.asarray(edge_weight, dtype=np.float32)
    edge_src = np.asarray(edge_src, dtype=np.int64)
    edge_dst = np.asarray(edge_dst, dtype=np.int64)

    # x^T padded, fp16
    xT = np.zeros((F0, NPAD), dtype=np.float16)
    xT[:, :N] = x.T.astype(np.float16)

    # dense transposed adjacency AT[src, dst] = sum of w over parallel edges
    AT = np.zeros((NPAD, NPAD), dtype=np.float32)
    np.add.at(AT, (edge_src, edge_dst), edge_weight)
    AT16 = AT.astype(np.float16)
    del AT

    in_maps = []
    common = {
        "xT": xT,
        "W1": W1.astype(np.float16),
        "W2": W2.astype(np.float16),
        "b1": b1.reshape(F1, 1).astype(np.float32),
        "b2": b2.reshape(F2, 1).astype(np.float32),
    }
    for c in range(NCORES):
        zsel = np.zeros((F2, NCORES), np.int32)
        for f in range(F2):
            for p in range(NCORES):
                zsel[f, p] = ((c + p) % NCORES) * F2 + f
        atc = AT16[:, c * SHARD:(c + 1) * SHARD]  # [NPAD, SHARD]
        # ATB: src perm M1 (node = k*128 + p), slab-major layout
        slabs_b = []
        slabs_d = []
        # M2 perm: partition p = 16r + a holds nodes 1280r + 80a + k
        atc_m2 = atc.reshape(P, KC, SHARD)  # [p, k, dst] with node = p*80+k
        for (n_off, nt) in NT:
            blkn = atc[:, n_off:n_off + nt]
            for kb in range(NSLABS):
                blk = blkn[kb * KSLAB * P:(kb + 1) * KSLAB * P, :]
                slabs_b.append(
                    blk.reshape(KSLAB, P, nt).transpose(1, 0, 2).reshape(P, -1))
                slabs_d.append(
                    atc_m2[:, kb * KSLAB:(kb + 1) * KSLAB, n_off:n_off + nt]
                    .reshape(P, -1))
        in_map = dict(common)
        in_map["zsel"] = zsel
        in_map["ATB"] = np.ascontiguousarray(np.concatenate(slabs_b, axis=1))
        in_map["ATD"] = np.ascontiguousarray(np.concatenate(slabs_d, axis=1))
        in_maps.append(in_map)
    return in_maps


def kernel(x, W1, b1, W2, b2, edge_weight, edge_src, edge_dst):
    from concourse import bass_utils

    nc = _get_program()
    in_maps = _host_prep(x, W1, b1, W2, b2, edge_weight, edge_src, edge_dst)
    res = bass_utils.run_bass_kernel_spmd(
        nc, in_maps, core_ids=list(range(NCORES)))
    z = np.concatenate(
        [res.results[c]["zT_out"].T for c in range(NCORES)], axis=0)[:N]

    M = np.zeros((NPAD, NPAD), dtype=np.float32)
    NBLK = NPAD // P  # 80
    for c in range(NCORES):
        ro = res.results[c]["recon_out"].astype(np.float32)  # [SHARD, WIN]
        for mt in range(SHARD // P):
            gI = c * (SHARD // P) + mt
            rows = slice(gI * P, (gI + 1) * P)
            strip = ro[mt * P:(mt + 1) * P, :]
            g0 = (gI * P) % NPAD
            if g0 + WIN <= NPAD:
                M[rows, g0:g0 + WIN] = strip
            else:
                k = NPAD - g0
                M[rows, g0:] = strip[:, :k]
                M[rows, :WIN - k] = strip[:, k:]
    # mirror the circulant upper coverage onto the lower blocks
    Mb = M.reshape(NBLK, P, NBLK, P)
    Is, Js = [], []
    for I in range(NBLK):
        for dd in range(1, DBLK):
            Is.append(I)
            Js.append((I + dd) % NBLK)
    Isa = np.asarray(Is)
    Jsa = np.asarray(Js)
    Mb[Jsa, :, Isa, :] = Mb[Isa, :, Jsa, :].transpose(0, 2, 1)
    recon = M[:N, :N]
    return z.astype(np.float32), np.ascontiguousarray(recon)
